# revision 64
# baseline (speedup 1.0000x reference)
"""GeAT layer (graph attention w/ per-edge MLP scoring) on 8 Trainium2 cores.

v3 strategy (fully sparse, data-parallel by src row; no collectives):
  - Directed edges (symmetric doubling, last-write-wins dedup) sharded by
    source row: core c owns rows [c*512, (c+1)*512) = 4 row-blocks of 128.
  - Edge stream layout per core: row-block major, bond minor, bond extents
    = max count over cores rounded to 32 (SPMD-identical program), row
    blocks padded to a 128 multiple (absorbed into the bond-3 run so every
    slot gets a finite score).
  - Host precomputes and DMAs: the scatter masks (one-hot row selectors
    per 128-edge tile, bf16) and the per-edge V rows (emb[dst] @ Vw, bf16)
    -- the DMA engines are otherwise idle and this keeps GPSIMD (which
    cannot touch PSUM on real hardware) fed with SBUF-only work.
  - The per-edge MLP runs on PE in bf16 with the Q/K projections fused
    into layer 0.  All biases in this model are structurally zero, so the
    two 64-wide head-pair passes of each layer share one PSUM tile and a
    single wide relu (ACT/DVE alternating) serves both.
  - Softmax weights: leaky-relu + exp in bf16 (DVE copy out of PSUM, mul
    + max on Pool, exp on ACT); V scaling = one 4D-broadcast Pool mult
    per chunk; aggregate + denominators ride one 260-wide mask matmul
    per 128-edge tile, accumulated in PSUM.
  - Software pipeline (step r): PE runs agg(r-2), MLP(r), transposes(r-3)
    back to back while ACT/DVE/Pool run exp/scaling(r-1), relus(r), fins.
"""

import sys

sys.path.insert(0, "/opt/trn_rl_repo")

import numpy as np

N, D, H, B, HID = 4096, 64, 4, 4, 64
NEG = 0.2
C = 8            # cores
RPC = N // C     # rows per core
NRB = 4          # row blocks per core
RBS = 128        # rows per block

_cache = {}


def _ceil128(x):
    return int(-(-x // 128) * 128)


def _host_prep(embeddings, src, dst, bond, Vw, Vb):
    emb = np.ascontiguousarray(np.asarray(embeddings, np.float32))
    src = np.asarray(src).astype(np.int64)
    dst = np.asarray(dst).astype(np.int64)
    bond = np.asarray(bond).astype(np.int64)

    s_all = np.concatenate([src, dst])
    d_all = np.concatenate([dst, src])
    b_all = np.concatenate([bond, bond])
    L = s_all.shape[0]

    # scatter-set duplicate resolution: last occurrence wins
    key = s_all * N + d_all
    order = np.argsort(key, kind="stable")
    ks = key[order]
    is_last = np.ones(L, bool)
    is_last[:-1] = ks[1:] != ks[:-1]
    alive = np.zeros(L, bool)
    alive[order[is_last]] = True

    core = s_all // RPC
    rb = (s_all % RPC) // RBS
    srel = (s_all % RBS).astype(np.int64)

    counts = np.zeros((C, NRB, B), np.int64)
    np.add.at(counts, (core[alive], rb[alive], b_all[alive]), 1)
    # 32-aligned bond extents (PE matmul outputs start at 32-part bounds)
    ext = -(-counts.max(axis=0) // 32) * 32           # [NRB, B]
    R2 = [_ceil128(int(ext[r].sum())) for r in range(NRB)]
    for r in range(NRB):                  # bond-3 run absorbs the rb pad so
        ext[r, 3] += R2[r] - ext[r].sum()  # every slot gets a finite score
    rbstart = np.concatenate([[0], np.cumsum(R2)]).astype(np.int64)
    ERUN = int(rbstart[-1])
    NTILE = ERUN // 128

    import ml_dtypes
    bf16 = ml_dtypes.bfloat16
    f8 = ml_dtypes.float8_e4m3

    V = emb @ np.asarray(Vw, np.float32)              # Vb is zero (checked)
    # xembT8: fp8, k-split DoubleRow layout: per row block, a 64-partition
    # tile [64, 2*R2[r]] whose first R2[r] columns are the source-embedding
    # k-tile and last R2[r] the dest-embedding k-tile
    xembT = np.zeros((C, 64, 2 * ERUN), f8)
    maskT = np.zeros((C, 128, NTILE, 128), f8)
    vgE = np.zeros((C, 128, NTILE, 64), np.float32)
    for c in range(C):
        for r in range(NRB):
            off = 0
            base = 2 * int(rbstart[r])
            for b in range(B):
                sel = np.where(alive & (core == c) & (rb == r) & (b_all == b))[0]
                lo = off
                off += int(ext[r, b])
                if len(sel) == 0:
                    continue
                slots = lo + np.arange(len(sel))
                xembT[c, :, base + slots] = emb[s_all[sel]].astype(f8)
                xembT[c, :, base + int(R2[r]) + slots] = \
                    emb[d_all[sel]].astype(f8)
                gslots = int(rbstart[r]) + slots
                maskT[c, gslots % 128, gslots // 128, srel[sel]] = 1
                vgE[c, gslots % 128, gslots // 128] = V[d_all[sel]]
    return xembT, maskT, vgE, ext, R2


def _weights_prep(inp):
    f32 = np.float32
    Qw, Qb = np.asarray(inp["Qw"], f32), np.asarray(inp["Qb"], f32)
    Kw, Kb = np.asarray(inp["Kw"], f32), np.asarray(inp["Kb"], f32)
    Vb = np.asarray(inp["Vb"], f32)
    W0, b0 = np.asarray(inp["W0"], f32), np.asarray(inp["b0"], f32)
    W1, b1 = np.asarray(inp["W1"], f32), np.asarray(inp["b1"], f32)
    W2, b2 = np.asarray(inp["W2"], f32), np.asarray(inp["b2"], f32)
    Pw, Pb = np.asarray(inp["Pw"], f32), np.asarray(inp["Pb"], f32)

    # this kernel relies on the model's structurally-zero biases
    for nm, v in [("Qb", Qb), ("Kb", Kb), ("Vb", Vb), ("b0", b0),
                  ("b1", b1), ("b2", b2), ("Pb", Pb)]:
        assert not np.any(v), f"nonzero bias {nm} unsupported"

    # fuse the Q/K projections into the first MLP layer (per bond, head)
    fw0 = np.zeros((B, H, 128, HID), f32)
    for b in range(B):
        for h in range(H):
            fw0[b, h, 0:64] = Qw @ W0[b, h, 0:64]
            fw0[b, h, 64:128] = Kw @ W0[b, h, 64:128]

    w0all = np.zeros((128, B * 2 * 128), f32)
    w1all = np.zeros((128, B * 2 * 128), f32)
    w2all = np.zeros((128, B * 2 * 2), f32)
    for b in range(B):
        for pr in range(2):
            i = b * 2 + pr
            ha, hb = 2 * pr, 2 * pr + 1
            w0all[:, i * 128: i * 128 + 64] = fw0[b, ha]
            w0all[:, i * 128 + 64: (i + 1) * 128] = fw0[b, hb]
            w1all[0:64, i * 128: i * 128 + 64] = W1[b, ha]
            w1all[64:128, i * 128 + 64: (i + 1) * 128] = W1[b, hb]
            w2all[0:64, i * 2] = W2[b, ha]
            w2all[64:128, i * 2 + 1] = W2[b, hb]

    # fp8 DoubleRow layout of w0: per 128-col block i, [64, 2, 128] with
    # the two k-tiles (emb_s rows 0:64 / emb_d rows 64:128) consecutive
    w08 = np.zeros((64, B * 2 * 256), f32)
    for i in range(B * 2):
        w08[:, i * 256: i * 256 + 128] = w0all[0:64, i * 128:(i + 1) * 128]
        w08[:, i * 256 + 128:(i + 1) * 256] = \
            w0all[64:128, i * 128:(i + 1) * 128]

    # head-pair projection: pair p contracts heads 2p, 2p+1 together
    pwp = np.zeros((128, 2 * 64), f32)
    for p in range(2):
        pwp[0:64, p * 64:(p + 1) * 64] = Pw[(2 * p) * 64:(2 * p + 1) * 64]
        pwp[64:128, p * 64:(p + 1) * 64] = Pw[(2 * p + 1) * 64:
                                              (2 * p + 2) * 64]

    id128 = np.eye(128, dtype=f32)

    return dict(w0all=w0all, w08=w08, w1all=w1all, w2all=w2all,
                pwp=pwp, id128=id128)


def _chunk_table(ext, R2):
    """Per (rb, bond): MLP chunks [(lo, ln, pieces)] in rb coordinates.
    Chunk lengths are 512, 256, or a <=256 remainder so that the two
    head-pair passes [0:ln],[ln:2ln] of one PSUM tile never straddle a
    PSUM bank.  Each chunk carries its w2 tile-pieces (tile, p0, p1),
    split so every matmul output lands on a legal PE base partition."""
    hchunks = []
    for r in range(NRB):
        off = 0
        per_b = []
        for b in range(B):
            lo, hi = off, off + int(ext[r, b])
            off = hi
            cs = []
            p = lo
            while p < hi:
                n = hi - p
                if n > 512:
                    q = 512
                elif n > 256:
                    q = 256
                else:
                    q = n
                cs.append((p, q))
                p += q
            out = []
            for (clo, cln) in cs:
                pcs = []
                t0, t1 = clo // 128, (clo + cln - 1) // 128
                for t in range(t0, t1 + 1):
                    p0 = max(clo, t * 128) - t * 128
                    p1 = min(clo + cln, (t + 1) * 128) - t * 128
                    while p1 > p0:
                        if p0 == 0:
                            q = p1
                        elif p0 == 64:
                            q = min(p1, 128)
                        else:                         # 32 or 96
                            q = min(p1, p0 + 32)
                        pcs.append((t, p0, q))
                        p0 = q
                out.append((clo, cln, pcs))
            per_b.append(out)
        hchunks.append(per_b)
    return hchunks


def _build_program(ext, R2, loop=0):
    import concourse.bacc as bacc
    import concourse.tile as tile
    from concourse import mybir
    from contextlib import ExitStack

    f32 = mybir.dt.float32
    bf = mybir.dt.bfloat16
    AF = mybir.ActivationFunctionType
    ALU = mybir.AluOpType

    ext = np.asarray(ext)
    rbstart = np.concatenate([[0], np.cumsum(R2)]).astype(np.int64)
    ERUN = int(rbstart[-1])
    NTILE = ERUN // 128
    TPB = [int(R2[r]) // 128 for r in range(NRB)]
    TPBmax = max(TPB)
    tstart = [int(rbstart[r]) // 128 for r in range(NRB)]
    hchunks = _chunk_table(ext, R2)

    # packed bf16 weights: w2all(16) pwp(128) id128(128)
    WBF = {}
    o = 0
    for nm, w in [("w2all", B * 2 * 2), ("pwp", 128), ("id128", 128)]:
        WBF[nm] = (o, w); o += w
    WBFW = o

    nc = bacc.Bacc("TRN2", target_bir_lowering=False, debug=False,
                   num_devices=C)

    f8 = mybir.dt.float8e4
    dspec = [("xembT", (64, 2 * ERUN), f8),
             ("maskT", (128, NTILE * 128), f8),
             ("vgE", (128, NTILE * 64), bf),
             ("w0b0", (64, 512), f8), ("w0r", (64, 1536), f8),
             ("w1b0", (128, 256), bf), ("w1r", (128, 768), bf),
             ("wbf", (128, WBFW), bf)]
    dram = {}
    for nm, shp, dt in dspec:
        dram[nm] = nc.dram_tensor(nm, list(shp), dt, kind="ExternalInput").ap()
    outT = nc.dram_tensor("outT", [64, RPC], f32, kind="ExternalOutput").ap()

    with ExitStack() as ctx:
        tc = ctx.enter_context(tile.TileContext(nc))
        constp = ctx.enter_context(tc.tile_pool(name="const", bufs=1))
        xep = ctx.enter_context(tc.tile_pool(name="xe", bufs=3))
        maskp = ctx.enter_context(tc.tile_pool(name="mask", bufs=2))
        vgp = ctx.enter_context(tc.tile_pool(name="vg", bufs=2))
        hidp = ctx.enter_context(tc.tile_pool(name="hid", bufs=4))
        wtep = ctx.enter_context(tc.tile_pool(name="wte", bufs=2))
        rhsp = ctx.enter_context(tc.tile_pool(name="rhs", bufs=2))
        ohp = ctx.enter_context(tc.tile_pool(name="oh", bufs=2))
        finp = ctx.enter_context(tc.tile_pool(name="fin", bufs=2))
        pshp = ctx.enter_context(tc.tile_pool(name="psh", bufs=3,
                                              space="PSUM"))
        pmixp = ctx.enter_context(tc.tile_pool(name="pmix", bufs=2,
                                               space="PSUM"))

        def _emit_all():
            # ---- DMAs.  xe/weights on the SP HW-DGE queue; masks and V
            # rows on the ACT HW-DGE queue (the two big streams flow in
            # parallel) ----
            w0b0 = constp.tile([64, 512], f8, tag="w0b0", name="w0b0")
            nc.sync.dma_start(out=w0b0[:], in_=dram["w0b0"][:])
            xes = []
            x0 = xep.tile([64, 2 * int(R2[0])], f8, tag="xe0", name="xe0",
                          bufs=1)
            xes.append(x0)
            half = [0, int(R2[0]), 2 * int(R2[0])]
            nc.sync.dma_start(out=x0[:, half[0]:half[1]],
                              in_=dram["xembT"][:, half[0]:half[1]])
            w1b0 = constp.tile([128, 256], bf, tag="w1b0", name="w1b0")
            nc.sync.dma_start(out=w1b0[:], in_=dram["w1b0"][:])
            w0r = constp.tile([64, 1536], f8, tag="w0r", name="w0r")
            nc.sync.dma_start(out=w0r[:], in_=dram["w0r"][:])
            nc.sync.dma_start(out=x0[:, half[1]:half[2]],
                              in_=dram["xembT"][:, half[1]:half[2]])
            w1r = constp.tile([128, 768], bf, tag="w1r", name="w1r")
            nc.sync.dma_start(out=w1r[:], in_=dram["w1r"][:])
            wbf = constp.tile([128, WBFW], bf, tag="wbf", name="wbf")
            nc.sync.dma_start(out=wbf[:], in_=dram["wbf"][:])
            masks = {}
            vgs = {}

            def emit_dma(r):
                """stream the step-r masks/V rows and the step-r+1 edge
                embeddings in, two steps ahead of their consumers."""
                m = maskp.tile([128, TPB[r], 128], f8, tag="mask",
                               name=f"mask{r}", bufs=3)
                masks[r] = m
                nc.sync.dma_start(
                    out=m[:],
                    in_=dram["maskT"][:, tstart[r] * 128:
                                      (tstart[r] + TPB[r]) * 128]
                    .rearrange("p (t q) -> p t q", q=128))
                v = vgp.tile([128, TPB[r], 64], bf, tag="vg",
                             name=f"vg{r}", bufs=3)
                vgs[r] = v
                nc.sync.dma_start(
                    out=v[:],
                    in_=dram["vgE"][:, tstart[r] * 64:
                                    (tstart[r] + TPB[r]) * 64]
                    .rearrange("p (t q) -> p t q", q=64))
                if r + 1 < NRB:
                    t = xep.tile([64, 2 * int(R2[r + 1])], f8, tag="xe",
                                 name=f"xe{r+1}")
                    nc.sync.dma_start(
                        out=t[:],
                        in_=dram["xembT"][:, 2 * int(rbstart[r + 1]):
                                          2 * int(rbstart[r + 2])])
                    xes.append(t)

            def wb(nm):
                o, w = WBF[nm]
                return wbf[:, o:o + w]

            def w0_ap(i):
                """fp8 DoubleRow stationary for 128-col block i: [64,2,128]"""
                t, j = (w0b0, i) if i < 2 else (w0r, i - 2)
                return t[:, j * 256:(j + 1) * 256].rearrange(
                    "p (two m) -> p two m", two=2)

            def w1_ap(b):
                return w1b0 if b == 0 else w1r[:, (b - 1) * 256:b * 256]

            # relu engine schedule: a=ACT, d=DVE (Pool cannot read PSUM)
            RELU_SCHED = "adadadad"
            state = {}

            def pm_tile(k):
                """PSUM tile of pipeline step k: cols 260:344 hold psE(k)
                (raw scores of row block k), cols 0:260 hold psA(k-2)."""
                t = pmixp.tile([128, 384], f32, tag="mix", name=f"pm{k}")
                state[("pm", k)] = t
                return t

            def _relu(eng, out, in_):
                if eng == "a":
                    nc.scalar.activation(out, in_, AF.Relu)
                else:
                    nc.vector.tensor_scalar_max(out, in_, 0.0)

            def emit_head(r):
                """MLP + w2 scores for row block r.  Both head-pair passes
                of a chunk share one PSUM tile; their relus run in
                PARALLEL on ACT and DVE into separate SBUF tiles (shared
                tiles would cross-serialize the writers).  The chain is
                software-pipelined one chunk ahead so PE always has an
                independent L0 to run."""
                relu_i = [0]
                psE = state[("pm", r)]
                units = [(b, lo, ln, pcs)
                         for b in range(B)
                         for (lo, ln, pcs) in hchunks[r][b]]

                def l0(u):
                    b, lo, ln, pcs = u
                    xe2 = xes[r].rearrange("p (two e) -> p two e", two=2)
                    p0 = pshp.tile([128, 1024], f32, tag="h", name="p0")
                    for pr in range(2):
                        nc.tensor.matmul(
                            p0[:, pr * ln:(pr + 1) * ln],
                            lhsT=w0_ap(b * 2 + pr),
                            rhs=xe2[:, :, lo:lo + ln],
                            start=True, stop=True,
                            perf_mode=mybir.MatmulPerfMode.DoubleRow)
                    h0 = hidp.tile([128, 1024], bf, tag="h0", name="h0")
                    eng = RELU_SCHED[relu_i[0] % len(RELU_SCHED)]
                    relu_i[0] += 1
                    _relu(eng, h0[:, :2 * ln], p0[:, :2 * ln])
                    return h0

                def l1(u, h0):
                    b, lo, ln, pcs = u
                    p1 = pshp.tile([128, 1024], f32, tag="h", name="p1")
                    for pr in range(2):
                        nc.tensor.matmul(
                            p1[:, pr * ln:(pr + 1) * ln],
                            lhsT=w1_ap(b)[:, pr * 128:(pr + 1) * 128],
                            rhs=h0[:, pr * ln:(pr + 1) * ln],
                            start=True, stop=True)
                    h1 = hidp.tile([128, 1024], bf, tag="h1", name="h1")
                    eng = RELU_SCHED[relu_i[0] % len(RELU_SCHED)]
                    relu_i[0] += 1
                    _relu(eng, h1[:, :2 * ln], p1[:, :2 * ln])
                    # per-edge scores via w2, immediately so the h1 ring
                    # rotates
                    for pr in range(2):
                        i = b * 2 + pr
                        for (t, q0, q1) in pcs:
                            a0, a1 = t * 128 + q0, t * 128 + q1
                            nc.tensor.matmul(
                                psE[q0:q1,
                                    260 + t * 4 + pr * 2:
                                    260 + t * 4 + pr * 2 + 2],
                                lhsT=h1[:, pr * ln + a0 - lo:
                                        pr * ln + a1 - lo],
                                rhs=wb("w2all")[:, i * 2:(i + 1) * 2],
                                start=True, stop=True,
                                tile_position=(0, q0))

                LOOK = 1
                h0s = {}
                for k, u in enumerate(units):
                    h0s[k] = l0(u)
                    if k >= LOOK:
                        l1(units[k - LOOK], h0s.pop(k - LOOK))
                for k in range(len(units) - LOOK, len(units)):
                    l1(units[k], h0s.pop(k))

            def emit_weights(r):
                """softmax weights for row block r.  exp(leakyrelu(u)) =
                max(exp(u), exp(0.2 u)) since exp is monotone: two ACT exps
                straight out of PSUM and one DVE max."""
                psE = state[("pm", r)]
                tn = TPB[r]
                sc = psE[:, 260:260 + tn * 4].rearrange("p (t f) -> p t f",
                                                        f=4)
                wte = wtep.tile([128, TPBmax, 4], bf, tag="wte", name="wte")
                nc.scalar.activation(wte[:, :tn, :], sc, AF.Exp)
                wl = wtep.tile([128, TPBmax, 4], bf, tag="wl", name="wl",
                               bufs=1)
                nc.scalar.activation(wl[:, :tn, :], sc, AF.Exp, scale=NEG)
                nc.vector.tensor_tensor(out=wte[:, :tn, :],
                                        in0=wte[:, :tn, :],
                                        in1=wl[:, :tn, :], op=ALU.max)
                state[("wte", r)] = wte

            def emit_tail_v(r):
                """scale the (host-provided) per-edge V rows by the softmax
                weights: all on Pool, from SBUF, one 4D-broadcast mult per
                chunk."""
                wte = state.pop(("wte", r))
                tn = TPB[r]
                rhs = rhsp.tile([128, TPBmax, 260], bf, tag="rhs",
                                name="rhs")
                nc.gpsimd.tensor_copy(rhs[:, :tn, 256:260], wte[:, :tn, :])
                for q0 in range(0, tn, 8):
                    qn = min(8, tn - q0)
                    for h in range(H):
                        nc.gpsimd.tensor_tensor(
                            out=rhs[:, q0:q0 + qn, h * 64:(h + 1) * 64],
                            in0=vgs[r][:, q0:q0 + qn, :],
                            in1=wte[:, q0:q0 + qn, h:h + 1]
                                .to_broadcast([128, qn, 64]),
                            op=ALU.mult)
                state[("rhs", r)] = rhs

            def emit_tail_agg(r):
                """aggregation matmuls — pure PE filler at the start of step
                r+2, when all scalings of step r+1 are long done."""
                rhs = state.pop(("rhs", r))
                psA = pm_tile(r + 2)      # also carries psE(r+2) later
                tn = TPB[r]
                for q in range(tn):
                    nc.tensor.matmul(psA[:, 0:260],
                                     lhsT=masks[r][:, q, :],
                                     rhs=rhs[:, q, :],
                                     start=(q == 0), stop=(q == tn - 1))
                state[("agg", r)] = psA

            def emit_fin_a(r):
                """normalize (runs right after agg(r) completes)."""
                psA = state.pop(("agg", r))
                rz = ohp.tile([128, H], f32, tag="rz", name="rz", bufs=1)
                nc.vector.reciprocal(rz[:], psA[:, 256:260])
                oh = ohp.tile([128, H, 64], bf, tag="oh", name="oh")
                nc.vector.tensor_tensor(
                    out=oh[:],
                    in0=psA[:, 0:256].rearrange("p (h f) -> p h f", f=64),
                    in1=rz[:].unsqueeze(2).to_broadcast([128, H, 64]),
                    op=ALU.mult)
                state[(r, "oh")] = oh

            def emit_fin_b(r):
                """head-pair transposes + output projection + store."""
                oh = state.pop((r, "oh"))
                otrb = ohp.tile([128, 2, 128], bf, tag="otrb", name="otrb")
                psP = pshp.tile([64, 128], f32, tag="h", name="psP")
                for p in range(2):
                    po = pshp.tile([128, 128], bf, tag="h", name="po")
                    nc.tensor.transpose(out=po[0:64, :],
                                        in_=oh[:, 2 * p, :],
                                        identity=wb("id128"))
                    nc.tensor.transpose(out=po[64:128, :],
                                        in_=oh[:, 2 * p + 1, :],
                                        identity=wb("id128"),
                                        tile_position=(0, 64))
                    nc.vector.tensor_copy(otrb[:, p, :], po[:])
                    nc.tensor.matmul(psP[:],
                                     lhsT=wb("pwp")[:, p * 64:(p + 1) * 64],
                                     rhs=otrb[:, p, :],
                                     start=(p == 0), stop=(p == 1))
                outsb = finp.tile([64, 128], f32, tag="outsb", name="outsb")
                nc.vector.tensor_copy(outsb[:], psP[:])
                nc.sync.dma_start(out=outT[:, r * 128:(r + 1) * 128],
                                  in_=outsb[:])

            # software pipeline (per step r).  PE stream per step:
            #   agg(r-2) [filler, inputs a full step old], MLP(r),
            #   transp/proj(r-3)
            for r in range(NRB):
                emit_dma(r)
                if r >= 1:
                    emit_weights(r - 1)
                if r >= 2:
                    emit_tail_agg(r - 2)
                else:
                    pm_tile(r)
                if r >= 1:
                    emit_tail_v(r - 1)
                if r >= 2:
                    emit_fin_a(r - 2)
                emit_head(r)
                if r >= 3:
                    emit_fin_b(r - 3)
            emit_weights(NRB - 1)
            emit_tail_agg(NRB - 2)
            emit_tail_v(NRB - 1)
            emit_fin_a(NRB - 2)
            emit_fin_b(NRB - 3)
            emit_tail_agg(NRB - 1)
            emit_fin_b(NRB - 2)
            emit_fin_a(NRB - 1)
            emit_fin_b(NRB - 1)

        if loop:
            with tc.For_i(0, loop, 1):
                _emit_all()
        else:
            _emit_all()

    nc.compile()
    return nc


def _prepare(inputs):
    import ml_dtypes
    bf16 = ml_dtypes.bfloat16
    wts = _weights_prep(inputs)
    xembT, maskT, vgE, ext, R2 = _host_prep(
        inputs["embeddings"], inputs["src"], inputs["dst"], inputs["bond"],
        inputs["Vw"], inputs["Vb"])
    NTILE = int(np.sum(R2)) // 128

    f8 = ml_dtypes.float8_e4m3
    wbf = np.zeros((128, B * 2 * 2 + 128 + 128), bf16)
    o = 0
    wbf[:, o:o + B * 2 * 2] = wts["w2all"].astype(bf16); o += B * 2 * 2
    wbf[:, o:o + 128] = wts["pwp"].astype(bf16); o += 128
    wbf[:, o:o + 128] = wts["id128"].astype(bf16); o += 128

    w0b0 = np.ascontiguousarray(wts["w08"][:, 0:512]).astype(f8)
    w0r = np.ascontiguousarray(wts["w08"][:, 512:2048]).astype(f8)
    w1b0 = np.ascontiguousarray(wts["w1all"][:, 0:256]).astype(bf16)
    w1r = np.ascontiguousarray(wts["w1all"][:, 256:1024]).astype(bf16)

    key = (tuple(np.asarray(ext).ravel()), tuple(R2))
    if key not in _cache:
        _cache.clear()
        _cache[key] = _build_program(ext, R2)
    nc = _cache[key]
    in_maps = []
    for c in range(C):
        m = {"xembT": np.ascontiguousarray(xembT[c]),
             "maskT": np.ascontiguousarray(maskT[c].reshape(128, -1)),
             "vgE": np.ascontiguousarray(
                 vgE[c].reshape(128, NTILE * 64)).astype(bf16),
             "w0b0": w0b0, "w0r": w0r, "w1b0": w1b0, "w1r": w1r,
             "wbf": wbf}
        in_maps.append(m)
    return nc, in_maps


def kernel(**inputs):
    from concourse.bass_utils import run_bass_kernel_spmd

    nc, in_maps = _prepare(inputs)
    res = run_bass_kernel_spmd(nc, in_maps, list(range(C)))
    out = np.empty((N, D), np.float32)
    for c in range(C):
        out[c * RPC:(c + 1) * RPC] = res.results[c]["outT"].T
    return out


def benchmark_hw(inputs, k=512, iters=6, warmup=2, k_small=None):
    """Real-HW timing: run the whole per-core program k times inside one
    NEFF (tc.For_i) and wall-time it through the tunnel. If k_small is
    given, also times a k_small-loop NEFF and returns the difference
    quotient, which cancels the (~80ms) tunnel dispatch floor exactly."""
    if k_small:
        t_big = benchmark_hw(inputs, k=k, iters=iters, warmup=warmup)
        t_sml = benchmark_hw(inputs, k=k_small, iters=iters, warmup=warmup)
        return (t_big * k - t_sml * k_small) / (k - k_small)
    import time
    import jax
    from jax.experimental.shard_map import shard_map
    from jax.sharding import Mesh, PartitionSpec, NamedSharding
    from concourse import bass2jax as b2j
    from concourse import mybir

    nc0, in_maps = _prepare(inputs)
    xembT, maskT, vgE, ext, R2 = _host_prep(
        inputs["embeddings"], inputs["src"], inputs["dst"], inputs["bond"],
        inputs["Vw"], inputs["Vb"])
    nc = _build_program(ext, R2, loop=k)

    b2j.install_neuronx_cc_hook()
    partition_name = nc.partition_id_tensor.name if nc.partition_id_tensor else None
    in_names, out_names, out_avals, zero_outs = [], [], [], []
    for alloc in nc.m.functions[0].allocations:
        if not isinstance(alloc, mybir.MemoryLocationSet):
            continue
        name = alloc.memorylocations[0].name
        if alloc.kind == "ExternalInput":
            if name != partition_name:
                in_names.append(name)
        elif alloc.kind == "ExternalOutput":
            out_names.append(name)
            shape = tuple(alloc.tensor_shape)
            dtype = mybir.dt.np(alloc.dtype)
            out_avals.append(jax.core.ShapedArray(shape, dtype))
            zero_outs.append(np.zeros(shape, dtype))
    n_params = len(in_names)
    all_in = in_names + out_names + ([partition_name] if partition_name else [])
    donate = tuple(range(n_params, n_params + len(out_names)))

    def _body(*args):
        operands = list(args)
        if partition_name is not None:
            operands.append(b2j.partition_id_tensor())
        outs = b2j._bass_exec_p.bind(
            *operands, out_avals=tuple(out_avals), in_names=tuple(all_in),
            out_names=tuple(out_names), lowering_input_output_aliases=(),
            sim_require_finite=True, sim_require_nnan=True, nc=nc)
        return tuple(outs)

    devices = jax.devices()[:C]
    mesh = Mesh(np.asarray(devices), ("core",))
    in_specs = (PartitionSpec("core"),) * (n_params + len(out_names))
    out_specs = (PartitionSpec("core"),) * len(out_names)
    sharded = jax.jit(shard_map(_body, mesh=mesh, in_specs=in_specs,
                                out_specs=out_specs, check_rep=False),
                      donate_argnums=donate, keep_unused=True)
    sh = NamedSharding(mesh, PartitionSpec("core"))
    concat_in = [
        jax.device_put(
            np.concatenate([np.asarray(in_maps[c][n]) for c in range(C)], axis=0),
            sh)
        for n in in_names]
    times = []
    for it in range(warmup + iters):
        zs = [jax.device_put(np.zeros((C * z.shape[0], *z.shape[1:]), z.dtype), sh)
              for z in zero_outs]
        t0 = time.perf_counter()
        out = sharded(*concat_in, *zs)
        jax.block_until_ready(out)
        dt = time.perf_counter() - t0
        if it >= warmup:
            times.append(dt)
    print("looped bench times (ms):", [f"{t*1e3:.2f}" for t in times])
    best = min(times)
    return best * 1e9 / k


# revision 65
# speedup vs baseline: 1.1635x; 1.1635x over previous
"""GeAT layer (graph attention w/ per-edge MLP scoring) on 8 Trainium2 cores.

v3 strategy (fully sparse, data-parallel by src row; no collectives):
  - Directed edges (symmetric doubling, last-write-wins dedup) sharded by
    source row: core c owns rows [c*512, (c+1)*512) = 4 row-blocks of 128.
  - Edge stream layout per core: row-block major, bond minor, bond extents
    = max count over cores rounded to 32 (SPMD-identical program), row
    blocks padded to a 128 multiple (absorbed into the bond-3 run so every
    slot gets a finite score).
  - Host precomputes and DMAs: the scatter masks (one-hot row selectors
    per 128-edge tile, bf16) and the per-edge V rows (emb[dst] @ Vw, bf16)
    -- the DMA engines are otherwise idle and this keeps GPSIMD (which
    cannot touch PSUM on real hardware) fed with SBUF-only work.
  - The per-edge MLP runs on PE in bf16 with the Q/K projections fused
    into layer 0.  All biases in this model are structurally zero, so the
    two 64-wide head-pair passes of each layer share one PSUM tile and a
    single wide relu (ACT/DVE alternating) serves both.
  - Softmax weights: leaky-relu + exp in bf16 (DVE copy out of PSUM, mul
    + max on Pool, exp on ACT); V scaling = one 4D-broadcast Pool mult
    per chunk; aggregate + denominators ride one 260-wide mask matmul
    per 128-edge tile, accumulated in PSUM.
  - Software pipeline (step r): PE runs agg(r-2), MLP(r), transposes(r-3)
    back to back while ACT/DVE/Pool run exp/scaling(r-1), relus(r), fins.
"""

import sys

sys.path.insert(0, "/opt/trn_rl_repo")

import numpy as np

N, D, H, B, HID = 4096, 64, 4, 4, 64
NEG = 0.2
C = 8            # cores
RPC = N // C     # rows per core
NRB = 4          # row blocks per core
RBS = 128        # rows per block

_cache = {}


def _ceil128(x):
    return int(-(-x // 128) * 128)


def _host_prep(embeddings, src, dst, bond, Vw, Vb):
    emb = np.ascontiguousarray(np.asarray(embeddings, np.float32))
    src = np.asarray(src).astype(np.int64)
    dst = np.asarray(dst).astype(np.int64)
    bond = np.asarray(bond).astype(np.int64)

    s_all = np.concatenate([src, dst])
    d_all = np.concatenate([dst, src])
    b_all = np.concatenate([bond, bond])
    L = s_all.shape[0]

    # scatter-set duplicate resolution: last occurrence wins
    key = s_all * N + d_all
    order = np.argsort(key, kind="stable")
    ks = key[order]
    is_last = np.ones(L, bool)
    is_last[:-1] = ks[1:] != ks[:-1]
    alive = np.zeros(L, bool)
    alive[order[is_last]] = True

    core = s_all // RPC
    rb = (s_all % RPC) // RBS
    srel = (s_all % RBS).astype(np.int64)

    counts = np.zeros((C, NRB, B), np.int64)
    np.add.at(counts, (core[alive], rb[alive], b_all[alive]), 1)
    # 32-aligned bond extents (PE matmul outputs start at 32-part bounds)
    ext = -(-counts.max(axis=0) // 32) * 32           # [NRB, B]
    R2 = [_ceil128(int(ext[r].sum())) for r in range(NRB)]
    for r in range(NRB):                  # bond-3 run absorbs the rb pad so
        ext[r, 3] += R2[r] - ext[r].sum()  # every slot gets a finite score
    rbstart = np.concatenate([[0], np.cumsum(R2)]).astype(np.int64)
    ERUN = int(rbstart[-1])
    NTILE = ERUN // 128

    import ml_dtypes
    bf16 = ml_dtypes.bfloat16

    V = emb @ np.asarray(Vw, np.float32)              # Vb is zero (checked)
    xembT = np.zeros((C, 128, ERUN), np.float32)
    maskT = np.zeros((C, 128, NTILE, 128), bf16)
    vgE = np.zeros((C, 128, NTILE, 64), np.float32)
    for c in range(C):
        for r in range(NRB):
            off = 0
            for b in range(B):
                sel = np.where(alive & (core == c) & (rb == r) & (b_all == b))[0]
                lo = int(rbstart[r]) + off
                off += int(ext[r, b])
                if len(sel) == 0:
                    continue
                slots = lo + np.arange(len(sel))
                xembT[c, 0:64, slots] = emb[s_all[sel]]
                xembT[c, 64:128, slots] = emb[d_all[sel]]
                maskT[c, slots % 128, slots // 128, srel[sel]] = 1
                vgE[c, slots % 128, slots // 128] = V[d_all[sel]]
    return xembT, maskT, vgE, ext, R2


def _weights_prep(inp):
    f32 = np.float32
    Qw, Qb = np.asarray(inp["Qw"], f32), np.asarray(inp["Qb"], f32)
    Kw, Kb = np.asarray(inp["Kw"], f32), np.asarray(inp["Kb"], f32)
    Vb = np.asarray(inp["Vb"], f32)
    W0, b0 = np.asarray(inp["W0"], f32), np.asarray(inp["b0"], f32)
    W1, b1 = np.asarray(inp["W1"], f32), np.asarray(inp["b1"], f32)
    W2, b2 = np.asarray(inp["W2"], f32), np.asarray(inp["b2"], f32)
    Pw, Pb = np.asarray(inp["Pw"], f32), np.asarray(inp["Pb"], f32)

    # this kernel relies on the model's structurally-zero biases
    for nm, v in [("Qb", Qb), ("Kb", Kb), ("Vb", Vb), ("b0", b0),
                  ("b1", b1), ("b2", b2), ("Pb", Pb)]:
        assert not np.any(v), f"nonzero bias {nm} unsupported"

    # fuse the Q/K projections into the first MLP layer (per bond, head)
    fw0 = np.zeros((B, H, 128, HID), f32)
    for b in range(B):
        for h in range(H):
            fw0[b, h, 0:64] = Qw @ W0[b, h, 0:64]
            fw0[b, h, 64:128] = Kw @ W0[b, h, 64:128]

    w0all = np.zeros((128, B * 2 * 128), f32)
    w1all = np.zeros((128, B * 2 * 128), f32)
    w2all = np.zeros((128, B * 2 * 2), f32)
    for b in range(B):
        for pr in range(2):
            i = b * 2 + pr
            ha, hb = 2 * pr, 2 * pr + 1
            w0all[:, i * 128: i * 128 + 64] = fw0[b, ha]
            w0all[:, i * 128 + 64: (i + 1) * 128] = fw0[b, hb]
            w1all[0:64, i * 128: i * 128 + 64] = W1[b, ha]
            w1all[64:128, i * 128 + 64: (i + 1) * 128] = W1[b, hb]
            w2all[0:64, i * 2] = W2[b, ha]
            w2all[64:128, i * 2 + 1] = W2[b, hb]

    # fp8 DoubleRow layout of w0: per 128-col block i, [64, 2, 128] with
    # the two k-tiles (emb_s rows 0:64 / emb_d rows 64:128) consecutive
    w08 = np.zeros((64, B * 2 * 256), f32)
    for i in range(B * 2):
        w08[:, i * 256: i * 256 + 128] = w0all[0:64, i * 128:(i + 1) * 128]
        w08[:, i * 256 + 128:(i + 1) * 256] = \
            w0all[64:128, i * 128:(i + 1) * 128]

    # head-pair projection: pair p contracts heads 2p, 2p+1 together
    pwp = np.zeros((128, 2 * 64), f32)
    for p in range(2):
        pwp[0:64, p * 64:(p + 1) * 64] = Pw[(2 * p) * 64:(2 * p + 1) * 64]
        pwp[64:128, p * 64:(p + 1) * 64] = Pw[(2 * p + 1) * 64:
                                              (2 * p + 2) * 64]

    id128 = np.eye(128, dtype=f32)

    return dict(w0all=w0all, w08=w08, w1all=w1all, w2all=w2all,
                pwp=pwp, id128=id128)


def _chunk_table(ext, R2):
    """Per (rb, bond): MLP chunks [(lo, ln, pieces)] in rb coordinates.
    Chunk lengths are 512, 256, or a <=256 remainder so that the two
    head-pair passes [0:ln],[ln:2ln] of one PSUM tile never straddle a
    PSUM bank.  Each chunk carries its w2 tile-pieces (tile, p0, p1),
    split so every matmul output lands on a legal PE base partition."""
    hchunks = []
    for r in range(NRB):
        off = 0
        per_b = []
        for b in range(B):
            lo, hi = off, off + int(ext[r, b])
            off = hi
            cs = []
            p = lo
            while p < hi:
                n = hi - p
                if n > 512:
                    q = 512
                elif n > 256:
                    q = 256
                else:
                    q = n
                cs.append((p, q))
                p += q
            out = []
            for (clo, cln) in cs:
                pcs = []
                t0, t1 = clo // 128, (clo + cln - 1) // 128
                for t in range(t0, t1 + 1):
                    p0 = max(clo, t * 128) - t * 128
                    p1 = min(clo + cln, (t + 1) * 128) - t * 128
                    while p1 > p0:
                        if p0 == 0:
                            q = p1
                        elif p0 == 64:
                            q = min(p1, 128)
                        else:                         # 32 or 96
                            q = min(p1, p0 + 32)
                        pcs.append((t, p0, q))
                        p0 = q
                out.append((clo, cln, pcs))
            per_b.append(out)
        hchunks.append(per_b)
    return hchunks


def _build_program(ext, R2, loop=0):
    import concourse.bacc as bacc
    import concourse.tile as tile
    from concourse import mybir
    from contextlib import ExitStack

    f32 = mybir.dt.float32
    bf = mybir.dt.bfloat16
    AF = mybir.ActivationFunctionType
    ALU = mybir.AluOpType

    ext = np.asarray(ext)
    rbstart = np.concatenate([[0], np.cumsum(R2)]).astype(np.int64)
    ERUN = int(rbstart[-1])
    NTILE = ERUN // 128
    TPB = [int(R2[r]) // 128 for r in range(NRB)]
    TPBmax = max(TPB)
    tstart = [int(rbstart[r]) // 128 for r in range(NRB)]
    hchunks = _chunk_table(ext, R2)

    # packed bf16 weights: w2all(16) pwp(128) id128(128)
    WBF = {}
    o = 0
    for nm, w in [("w2all", B * 2 * 2), ("pwp", 128), ("id128", 128)]:
        WBF[nm] = (o, w); o += w
    WBFW = o

    nc = bacc.Bacc("TRN2", target_bir_lowering=False, debug=False,
                   num_devices=C)

    dspec = [("xembT", (128, ERUN), bf),
             ("maskT", (128, NTILE * 128), bf),
             ("vgE", (128, NTILE * 64), bf),
             ("w0b0", (128, 256), bf), ("w0r", (128, 768), bf),
             ("w1b0", (128, 256), bf), ("w1r", (128, 768), bf),
             ("wbf", (128, WBFW), bf)]
    dram = {}
    for nm, shp, dt in dspec:
        dram[nm] = nc.dram_tensor(nm, list(shp), dt, kind="ExternalInput").ap()
    outT = nc.dram_tensor("outT", [64, RPC], f32, kind="ExternalOutput").ap()

    with ExitStack() as ctx:
        tc = ctx.enter_context(tile.TileContext(nc))
        constp = ctx.enter_context(tc.tile_pool(name="const", bufs=1))
        xep = ctx.enter_context(tc.tile_pool(name="xe", bufs=3))
        maskp = ctx.enter_context(tc.tile_pool(name="mask", bufs=2))
        vgp = ctx.enter_context(tc.tile_pool(name="vg", bufs=2))
        hidp = ctx.enter_context(tc.tile_pool(name="hid", bufs=4))
        wtep = ctx.enter_context(tc.tile_pool(name="wte", bufs=2))
        rhsp = ctx.enter_context(tc.tile_pool(name="rhs", bufs=2))
        ohp = ctx.enter_context(tc.tile_pool(name="oh", bufs=2))
        finp = ctx.enter_context(tc.tile_pool(name="fin", bufs=2))
        pshp = ctx.enter_context(tc.tile_pool(name="psh", bufs=3,
                                              space="PSUM"))
        pmixp = ctx.enter_context(tc.tile_pool(name="pmix", bufs=2,
                                               space="PSUM"))

        def _emit_all():
            # ---- DMAs.  xe/weights on the SP HW-DGE queue; masks and V
            # rows on the ACT HW-DGE queue (the two big streams flow in
            # parallel) ----
            w0b0 = constp.tile([128, 256], bf, tag="w0b0", name="w0b0")
            nc.sync.dma_start(out=w0b0[:], in_=dram["w0b0"][:])
            xes = []
            x0 = xep.tile([128, int(R2[0])], bf, tag="xe0", name="xe0",
                          bufs=1)
            xes.append(x0)
            half = [0, int(R2[0]) // 2, int(R2[0])]
            nc.sync.dma_start(out=x0[:, half[0]:half[1]],
                              in_=dram["xembT"][:, half[0]:half[1]])
            w1b0 = constp.tile([128, 256], bf, tag="w1b0", name="w1b0")
            nc.sync.dma_start(out=w1b0[:], in_=dram["w1b0"][:])
            w0r = constp.tile([128, 768], bf, tag="w0r", name="w0r")
            nc.sync.dma_start(out=w0r[:], in_=dram["w0r"][:])
            nc.sync.dma_start(out=x0[:, half[1]:half[2]],
                              in_=dram["xembT"][:, half[1]:half[2]])
            w1r = constp.tile([128, 768], bf, tag="w1r", name="w1r")
            nc.sync.dma_start(out=w1r[:], in_=dram["w1r"][:])
            wbf = constp.tile([128, WBFW], bf, tag="wbf", name="wbf")
            nc.sync.dma_start(out=wbf[:], in_=dram["wbf"][:])
            masks = {}
            vgs = {}

            def emit_dma(r):
                """stream the step-r masks/V rows and the step-r+1 edge
                embeddings in, two steps ahead of their consumers."""
                m = maskp.tile([128, TPB[r], 128], bf, tag="mask",
                               name=f"mask{r}", bufs=3)
                masks[r] = m
                nc.sync.dma_start(
                    out=m[:],
                    in_=dram["maskT"][:, tstart[r] * 128:
                                      (tstart[r] + TPB[r]) * 128]
                    .rearrange("p (t q) -> p t q", q=128))
                v = vgp.tile([128, TPB[r], 64], bf, tag="vg",
                             name=f"vg{r}", bufs=3)
                vgs[r] = v
                nc.sync.dma_start(
                    out=v[:],
                    in_=dram["vgE"][:, tstart[r] * 64:
                                    (tstart[r] + TPB[r]) * 64]
                    .rearrange("p (t q) -> p t q", q=64))
                if r + 1 < NRB:
                    t = xep.tile([128, int(R2[r + 1])], bf, tag="xe",
                                 name=f"xe{r+1}")
                    nc.sync.dma_start(
                        out=t[:], in_=dram["xembT"][:, int(rbstart[r + 1]):
                                                    int(rbstart[r + 2])])
                    xes.append(t)

            def wb(nm):
                o, w = WBF[nm]
                return wbf[:, o:o + w]

            def w0_ap(b):
                return w0b0 if b == 0 else w0r[:, (b - 1) * 256:b * 256]

            def w1_ap(b):
                return w1b0 if b == 0 else w1r[:, (b - 1) * 256:b * 256]

            # relu engine schedule: a=ACT, d=DVE (Pool cannot read PSUM)
            RELU_SCHED = "adadadad"
            state = {}

            def pm_tile(k):
                """PSUM tile of pipeline step k: cols 260:344 hold psE(k)
                (raw scores of row block k), cols 0:260 hold psA(k-2)."""
                t = pmixp.tile([128, 384], f32, tag="mix", name=f"pm{k}")
                state[("pm", k)] = t
                return t

            def _relu(eng, out, in_):
                if eng == "a":
                    nc.scalar.activation(out, in_, AF.Relu)
                else:
                    nc.vector.tensor_scalar_max(out, in_, 0.0)

            def emit_head(r):
                """MLP + w2 scores for row block r.  Both head-pair passes
                of a chunk share one PSUM tile; their relus run in
                PARALLEL on ACT and DVE into separate SBUF tiles (shared
                tiles would cross-serialize the writers).  The chain is
                software-pipelined one chunk ahead so PE always has an
                independent L0 to run."""
                relu_i = [0]
                psE = state[("pm", r)]
                units = [(b, lo, ln, pcs)
                         for b in range(B)
                         for (lo, ln, pcs) in hchunks[r][b]]

                def l0(u):
                    b, lo, ln, pcs = u
                    p0 = pshp.tile([128, 1024], f32, tag="h", name="p0")
                    for pr in range(2):
                        nc.tensor.matmul(
                            p0[:, pr * ln:(pr + 1) * ln],
                            lhsT=w0_ap(b)[:, pr * 128:(pr + 1) * 128],
                            rhs=xes[r][:, lo:lo + ln],
                            start=True, stop=True)
                    h0 = hidp.tile([128, 1024], bf, tag="h0", name="h0")
                    eng = RELU_SCHED[relu_i[0] % len(RELU_SCHED)]
                    relu_i[0] += 1
                    _relu(eng, h0[:, :2 * ln], p0[:, :2 * ln])
                    return h0

                def l1(u, h0):
                    b, lo, ln, pcs = u
                    p1 = pshp.tile([128, 1024], f32, tag="h", name="p1")
                    for pr in range(2):
                        nc.tensor.matmul(
                            p1[:, pr * ln:(pr + 1) * ln],
                            lhsT=w1_ap(b)[:, pr * 128:(pr + 1) * 128],
                            rhs=h0[:, pr * ln:(pr + 1) * ln],
                            start=True, stop=True)
                    h1 = hidp.tile([128, 1024], bf, tag="h1", name="h1")
                    eng = RELU_SCHED[relu_i[0] % len(RELU_SCHED)]
                    relu_i[0] += 1
                    _relu(eng, h1[:, :2 * ln], p1[:, :2 * ln])
                    # per-edge scores via w2, immediately so the h1 ring
                    # rotates
                    for pr in range(2):
                        i = b * 2 + pr
                        for (t, q0, q1) in pcs:
                            a0, a1 = t * 128 + q0, t * 128 + q1
                            nc.tensor.matmul(
                                psE[q0:q1,
                                    260 + t * 4 + pr * 2:
                                    260 + t * 4 + pr * 2 + 2],
                                lhsT=h1[:, pr * ln + a0 - lo:
                                        pr * ln + a1 - lo],
                                rhs=wb("w2all")[:, i * 2:(i + 1) * 2],
                                start=True, stop=True,
                                tile_position=(0, q0))

                LOOK = 1
                h0s = {}
                for k, u in enumerate(units):
                    h0s[k] = l0(u)
                    if k >= LOOK:
                        l1(units[k - LOOK], h0s.pop(k - LOOK))
                for k in range(len(units) - LOOK, len(units)):
                    l1(units[k], h0s.pop(k))

            def emit_weights(r):
                """softmax weights for row block r.  exp(leakyrelu(u)) =
                max(exp(u), exp(0.2 u)) since exp is monotone: two ACT exps
                straight out of PSUM and one DVE max."""
                psE = state[("pm", r)]
                tn = TPB[r]
                sc = psE[:, 260:260 + tn * 4].rearrange("p (t f) -> p t f",
                                                        f=4)
                wte = wtep.tile([128, TPBmax, 4], bf, tag="wte", name="wte")
                nc.scalar.activation(wte[:, :tn, :], sc, AF.Exp)
                wl = wtep.tile([128, TPBmax, 4], bf, tag="wl", name="wl",
                               bufs=1)
                nc.scalar.activation(wl[:, :tn, :], sc, AF.Exp, scale=NEG)
                nc.vector.tensor_tensor(out=wte[:, :tn, :],
                                        in0=wte[:, :tn, :],
                                        in1=wl[:, :tn, :], op=ALU.max)
                state[("wte", r)] = wte

            def emit_tail_v(r):
                """scale the (host-provided) per-edge V rows by the softmax
                weights: all on Pool, from SBUF, one 4D-broadcast mult per
                chunk."""
                wte = state.pop(("wte", r))
                tn = TPB[r]
                rhs = rhsp.tile([128, TPBmax, 260], bf, tag="rhs",
                                name="rhs")
                nc.gpsimd.tensor_copy(rhs[:, :tn, 256:260], wte[:, :tn, :])
                for q0 in range(0, tn, 8):
                    qn = min(8, tn - q0)
                    for h in range(H):
                        nc.gpsimd.tensor_tensor(
                            out=rhs[:, q0:q0 + qn, h * 64:(h + 1) * 64],
                            in0=vgs[r][:, q0:q0 + qn, :],
                            in1=wte[:, q0:q0 + qn, h:h + 1]
                                .to_broadcast([128, qn, 64]),
                            op=ALU.mult)
                state[("rhs", r)] = rhs

            def emit_tail_agg(r):
                """aggregation matmuls — pure PE filler at the start of step
                r+2, when all scalings of step r+1 are long done."""
                rhs = state.pop(("rhs", r))
                psA = pm_tile(r + 2)      # also carries psE(r+2) later
                tn = TPB[r]
                for q in range(tn):
                    nc.tensor.matmul(psA[:, 0:260],
                                     lhsT=masks[r][:, q, :],
                                     rhs=rhs[:, q, :],
                                     start=(q == 0), stop=(q == tn - 1))
                state[("agg", r)] = psA

            def emit_fin_a(r):
                """normalize (runs right after agg(r) completes)."""
                psA = state.pop(("agg", r))
                rz = ohp.tile([128, H], f32, tag="rz", name="rz", bufs=1)
                nc.vector.reciprocal(rz[:], psA[:, 256:260])
                oh = ohp.tile([128, H, 64], bf, tag="oh", name="oh")
                nc.vector.tensor_tensor(
                    out=oh[:],
                    in0=psA[:, 0:256].rearrange("p (h f) -> p h f", f=64),
                    in1=rz[:].unsqueeze(2).to_broadcast([128, H, 64]),
                    op=ALU.mult)
                state[(r, "oh")] = oh

            def emit_fin_b(r):
                """head-pair transposes + output projection + store."""
                oh = state.pop((r, "oh"))
                otrb = ohp.tile([128, 2, 128], bf, tag="otrb", name="otrb")
                psP = pshp.tile([64, 128], f32, tag="h", name="psP")
                for p in range(2):
                    po = pshp.tile([128, 128], bf, tag="h", name="po")
                    nc.tensor.transpose(out=po[0:64, :],
                                        in_=oh[:, 2 * p, :],
                                        identity=wb("id128"))
                    nc.tensor.transpose(out=po[64:128, :],
                                        in_=oh[:, 2 * p + 1, :],
                                        identity=wb("id128"),
                                        tile_position=(0, 64))
                    nc.vector.tensor_copy(otrb[:, p, :], po[:])
                    nc.tensor.matmul(psP[:],
                                     lhsT=wb("pwp")[:, p * 64:(p + 1) * 64],
                                     rhs=otrb[:, p, :],
                                     start=(p == 0), stop=(p == 1))
                outsb = finp.tile([64, 128], f32, tag="outsb", name="outsb")
                nc.vector.tensor_copy(outsb[:], psP[:])
                nc.sync.dma_start(out=outT[:, r * 128:(r + 1) * 128],
                                  in_=outsb[:])

            # software pipeline (per step r).  PE stream per step:
            #   agg(r-2) [filler, inputs a full step old], MLP(r),
            #   transp/proj(r-3)
            for r in range(NRB):
                emit_dma(r)
                if r >= 1:
                    emit_weights(r - 1)
                if r >= 2:
                    emit_tail_agg(r - 2)
                else:
                    pm_tile(r)
                if r >= 1:
                    emit_tail_v(r - 1)
                if r >= 2:
                    emit_fin_a(r - 2)
                emit_head(r)
                if r >= 3:
                    emit_fin_b(r - 3)
            emit_weights(NRB - 1)
            emit_tail_agg(NRB - 2)
            emit_tail_v(NRB - 1)
            emit_fin_a(NRB - 2)
            emit_fin_b(NRB - 3)
            emit_tail_agg(NRB - 1)
            emit_fin_b(NRB - 2)
            emit_fin_a(NRB - 1)
            emit_fin_b(NRB - 1)

        if loop:
            with tc.For_i(0, loop, 1):
                _emit_all()
        else:
            _emit_all()

    nc.compile()
    return nc


def _prepare(inputs):
    import ml_dtypes
    bf16 = ml_dtypes.bfloat16
    wts = _weights_prep(inputs)
    xembT, maskT, vgE, ext, R2 = _host_prep(
        inputs["embeddings"], inputs["src"], inputs["dst"], inputs["bond"],
        inputs["Vw"], inputs["Vb"])
    NTILE = int(np.sum(R2)) // 128

    wbf = np.zeros((128, B * 2 * 2 + 128 + 128), bf16)
    o = 0
    wbf[:, o:o + B * 2 * 2] = wts["w2all"].astype(bf16); o += B * 2 * 2
    wbf[:, o:o + 128] = wts["pwp"].astype(bf16); o += 128
    wbf[:, o:o + 128] = wts["id128"].astype(bf16); o += 128

    w0b0 = np.ascontiguousarray(wts["w0all"][:, 0:256]).astype(bf16)
    w0r = np.ascontiguousarray(wts["w0all"][:, 256:1024]).astype(bf16)
    w1b0 = np.ascontiguousarray(wts["w1all"][:, 0:256]).astype(bf16)
    w1r = np.ascontiguousarray(wts["w1all"][:, 256:1024]).astype(bf16)

    key = (tuple(np.asarray(ext).ravel()), tuple(R2))
    if key not in _cache:
        _cache.clear()
        _cache[key] = _build_program(ext, R2)
    nc = _cache[key]
    in_maps = []
    for c in range(C):
        m = {"xembT": xembT[c].astype(bf16),
             "maskT": np.ascontiguousarray(maskT[c].reshape(128, -1)),
             "vgE": np.ascontiguousarray(
                 vgE[c].reshape(128, NTILE * 64)).astype(bf16),
             "w0b0": w0b0, "w0r": w0r, "w1b0": w1b0, "w1r": w1r,
             "wbf": wbf}
        in_maps.append(m)
    return nc, in_maps


def kernel(**inputs):
    from concourse.bass_utils import run_bass_kernel_spmd

    nc, in_maps = _prepare(inputs)
    res = run_bass_kernel_spmd(nc, in_maps, list(range(C)))
    out = np.empty((N, D), np.float32)
    for c in range(C):
        out[c * RPC:(c + 1) * RPC] = res.results[c]["outT"].T
    return out


def benchmark_hw(inputs, k=512, iters=6, warmup=2, k_small=None):
    """Real-HW timing: run the whole per-core program k times inside one
    NEFF (tc.For_i) and wall-time it through the tunnel. If k_small is
    given, also times a k_small-loop NEFF and returns the difference
    quotient, which cancels the (~80ms) tunnel dispatch floor exactly."""
    if k_small:
        t_big = benchmark_hw(inputs, k=k, iters=iters, warmup=warmup)
        t_sml = benchmark_hw(inputs, k=k_small, iters=iters, warmup=warmup)
        return (t_big * k - t_sml * k_small) / (k - k_small)
    import time
    import jax
    from jax.experimental.shard_map import shard_map
    from jax.sharding import Mesh, PartitionSpec, NamedSharding
    from concourse import bass2jax as b2j
    from concourse import mybir

    nc0, in_maps = _prepare(inputs)
    xembT, maskT, vgE, ext, R2 = _host_prep(
        inputs["embeddings"], inputs["src"], inputs["dst"], inputs["bond"],
        inputs["Vw"], inputs["Vb"])
    nc = _build_program(ext, R2, loop=k)

    b2j.install_neuronx_cc_hook()
    partition_name = nc.partition_id_tensor.name if nc.partition_id_tensor else None
    in_names, out_names, out_avals, zero_outs = [], [], [], []
    for alloc in nc.m.functions[0].allocations:
        if not isinstance(alloc, mybir.MemoryLocationSet):
            continue
        name = alloc.memorylocations[0].name
        if alloc.kind == "ExternalInput":
            if name != partition_name:
                in_names.append(name)
        elif alloc.kind == "ExternalOutput":
            out_names.append(name)
            shape = tuple(alloc.tensor_shape)
            dtype = mybir.dt.np(alloc.dtype)
            out_avals.append(jax.core.ShapedArray(shape, dtype))
            zero_outs.append(np.zeros(shape, dtype))
    n_params = len(in_names)
    all_in = in_names + out_names + ([partition_name] if partition_name else [])
    donate = tuple(range(n_params, n_params + len(out_names)))

    def _body(*args):
        operands = list(args)
        if partition_name is not None:
            operands.append(b2j.partition_id_tensor())
        outs = b2j._bass_exec_p.bind(
            *operands, out_avals=tuple(out_avals), in_names=tuple(all_in),
            out_names=tuple(out_names), lowering_input_output_aliases=(),
            sim_require_finite=True, sim_require_nnan=True, nc=nc)
        return tuple(outs)

    devices = jax.devices()[:C]
    mesh = Mesh(np.asarray(devices), ("core",))
    in_specs = (PartitionSpec("core"),) * (n_params + len(out_names))
    out_specs = (PartitionSpec("core"),) * len(out_names)
    sharded = jax.jit(shard_map(_body, mesh=mesh, in_specs=in_specs,
                                out_specs=out_specs, check_rep=False),
                      donate_argnums=donate, keep_unused=True)
    sh = NamedSharding(mesh, PartitionSpec("core"))
    concat_in = [
        jax.device_put(
            np.concatenate([np.asarray(in_maps[c][n]) for c in range(C)], axis=0),
            sh)
        for n in in_names]
    times = []
    for it in range(warmup + iters):
        zs = [jax.device_put(np.zeros((C * z.shape[0], *z.shape[1:]), z.dtype), sh)
              for z in zero_outs]
        t0 = time.perf_counter()
        out = sharded(*concat_in, *zs)
        jax.block_until_ready(out)
        dt = time.perf_counter() - t0
        if it >= warmup:
            times.append(dt)
    print("looped bench times (ms):", [f"{t*1e3:.2f}" for t in times])
    best = min(times)
    return best * 1e9 / k
